# revision 83
# baseline (speedup 1.0000x reference)
"""Single-head attention (SEQ=8192, D_MODEL=2048, D_K=128) on 8 TRN2 NeuronCores.

Sequence-parallel: each core owns 1024 query rows. K^T and V are computed
per local sequence quarter/half and all-gathered in bf16; attention runs
in S^T layout ([key, query] tiles) over two query-half passes of 32
double-key-block tiles each.

With collectives excluded from the timed model, the tensor engine is the
bottleneck (~77us of matmul work at 2.4GHz: projections 20.5, scores
27.3, P@V 27.3, denominators ~1) against the scalar engine's 66.4us exp
stream. The schedule:
- pins every input/gather DMA to an explicit SP-queue ladder (priority
  pins, not waits) ordered by data deadline: consts, wq, biasf, x-half0,
  kth quarter-0 loads, wk, wv, x-half1, then the remaining gather loads
  interleaved; in the timed (collective-free) model the gather loads have
  no producers, so only the Q0 projection gates exp-0 (~12.5us);
- emits ALL projections in phase A (every gather load precedes every
  score in emission order, keeping real-build collective dependencies
  sound) but pins their execution into tile-indexed bands that follow
  the ps_a PSUM pool's strict FIFO ring: K0 and K1 (K1 in the warm
  bank's pool) fill the early x-arrival window, Q1 finishes mid-pass-0,
  V0/V1 run inside pass 0 so their heavy staging (4 transposes + vd
  write) stays off the kernel tail, and K2/K3 absorb pass-1's slack with
  their light ktd staging on the then-idle Act queue;
- keeps bias-add/dtype-convert finishes off the scalar engine during the
  exp stream (vector engine), except K2/K3's which run on the Act engine
  after the stream ends so they can't stall the DVE tail chain;
- P@V lags its exp by 10 tiles in pass 0 / 4 in pass 1, pushing PV work
  into the pass-1 PE slack; the first scores of each pass carry priority
  boosts so the exp stream never gaps at the pass boundary;
- softmax denominators are partition-reduced on the idle GPSIMD engine
  (pass-0 fully; pass-1's bulk starts after tile 61 so the reduce+adds
  +reciprocal chain overlaps the last two exps, with the final two
  tiles' sums on the fast PE ones-matmul path), and pass-1's output
  accumulator lives in the warm bank's PSUM pool so its first P@V never
  waits on pass-0's multiply;
- ones-fed filler matmuls (lowest preference) hold the PE p-state at
  full speed through the DMA-gated prologue.
"""
import os

import numpy as np
import ml_dtypes

import concourse.bacc as bacc
import concourse.tile as tile
from concourse import mybir
from concourse import bass_isa
from concourse.bass_utils import run_bass_kernel_spmd

N_CORES = 8
SEQ = 8192
DM = 2048
DK = 128
SL = SEQ // N_CORES          # 1024 local rows
NMC = DM // 128              # 16 contraction chunks for projections
SCALE = float(np.sqrt(DK))

F32 = mybir.dt.float32
BF16 = mybir.dt.bfloat16
EXP = mybir.ActivationFunctionType.Exp
IDENT = mybir.ActivationFunctionType.Identity


def _build():
    nc = bacc.Bacc(
        "TRN2",
        target_bir_lowering=False,
        debug=False,
        num_devices=N_CORES,
    )

    # host-prepacked partition-major layouts for full-bandwidth loads
    xP = nc.dram_tensor("xP", [128, 4, NMC, 256], BF16,
                    kind="ExternalInput")
    wqP = nc.dram_tensor("wqP", [128, NMC, DK], BF16, kind="ExternalInput")
    wkP = nc.dram_tensor("wkP", [128, NMC, DK], BF16, kind="ExternalInput")
    wvP = nc.dram_tensor("wvP", [128, NMC, DK], BF16, kind="ExternalInput")
    # [:, 0:128] ones, [:, 128:131] bq/bk/bv biases
    consts_d = nc.dram_tensor("consts_d", [128, 132], BF16,
                              kind="ExternalInput")
    # fp32 biases for the DVE tensor_scalar ops: [:, 0]=bq [:, 1]=bk [:, 2]=bv
    biasf_d = nc.dram_tensor("biasf_d", [128, 3], F32, kind="ExternalInput")
    out = nc.dram_tensor("out", [DK, SL], F32, kind="ExternalOutput")

    skip_cc = os.environ.get("KCC", "") == "skip"
    groups = [list(range(N_CORES))]

    with tile.TileContext(nc) as tc:
        with (
            tc.tile_pool(name="const", bufs=1) as const_pool,
            tc.tile_pool(name="w", bufs=1) as w_pool,
            tc.tile_pool(name="proj", bufs=1) as proj_pool,
            tc.tile_pool(name="kv", bufs=1) as kv_pool,
            tc.tile_pool(name="pt", bufs=30) as pt_pool,
            tc.tile_pool(name="fin", bufs=1) as fin_pool,
            tc.tile_pool(name="dram", bufs=1, space="DRAM") as dram_pool,
            tc.tile_pool(name="ps_a", bufs=2, space="PSUM") as ps_a,
            tc.tile_pool(name="ps_st", bufs=2, space="PSUM") as ps_st,
            tc.tile_pool(name="ps_o", bufs=1, space="PSUM") as ps_o,
            tc.tile_pool(name="ps_w", bufs=1, space="PSUM") as ps_w,
        ):
            # ---- SBUF tiles ----
            consts = const_pool.tile([128, 132], BF16)
            ones_r = consts[:, 0:128]
            biasf = const_pool.tile([128, 3], F32)
            bq_sb = biasf[:, 0:1]
            bk_sb = biasf[:, 1:2]
            bv_sb = biasf[:, 2:3]

            wk_t = w_pool.tile([128, NMC, DK], BF16)
            wq_t = w_pool.tile([128, NMC, DK], BF16)
            wv_t = w_pool.tile([128, NMC, DK], BF16)
            wk_sb = [wk_t[:, i, :] for i in range(NMC)]
            wq_sb = [wq_t[:, i, :] for i in range(NMC)]
            wv_sb = [wv_t[:, i, :] for i in range(NMC)]
            # x quarters: [128, chunk, 256 seq]; quarter q = local seq
            # columns q*256..(q+1)*256 (all d_model chunks)
            x_q = [w_pool.tile([128, NMC, 256], BF16, name=f"xq{q}")
                   for q in range(4)]

            # local K^T quarters / Q^T / V^T halves (bf16, biased)
            kq_loc = [proj_pool.tile([128, 256], BF16, name=f"kql{q}")
                      for q in range(4)]
            qt_sb = [proj_pool.tile([128, 512], BF16, name=f"qtl{h}")
                     for h in range(2)]
            vt_h = [proj_pool.tile([128, 512], BF16, name=f"vth{h}")
                    for h in range(2)]
            vsb_h = [proj_pool.tile([128, 4, 128], BF16, name=f"vs{h}")
                     for h in range(2)]

            # gathered K^T (per source quarter) / V, split per staging
            # writer so readers don't serialize on unrelated DMAs
            kth_q = [[kv_pool.tile([128, nb, 256], BF16, name=f"kq{q}{i}")
                      for i, nb in enumerate((2, 6))] for q in range(4)]
            v_t = [kv_pool.tile([128, N_CORES, 128], BF16, name=f"van{ht}")
                   for ht in range(8)]

            # DRAM staging + gathered buffers
            ktd_q = [dram_pool.tile([128, 256], BF16, name=f"ktd{q}")
                     for q in range(4)]
            vd_h = [dram_pool.tile([512, DK], BF16, name=f"vd{h}")
                    for h in range(2)]
            ktg_q = [dram_pool.tile([N_CORES, 128, 256], BF16,
                                    addr_space="Shared", name=f"ktg{q}")
                     for q in range(4)]
            vg_h = [dram_pool.tile([N_CORES, 512, DK], BF16,
                                   addr_space="Shared", name=f"vg{h}")
                    for h in range(2)]

            # ---- input DMA ladder (SP queue) ----
            # Every SP-queue DMA gets an explicitly pinned priority so the
            # scheduler's queue order (= the FIFO the bus serves) is exactly
            # the ladder below, independent of emission point. Deadlines:
            # exp-0 needs wq+x(h0)+kth00; tile 8j needs kth quarter j; PV of
            # tile j (lag 8) needs its v_t blocks; xq2/3 feed K2/K3/Q1/V1.
            LP = -90000

            def pin(i):
                return tc.high_priority(offset=tc.cur_priority - (LP + i))

            def spdma(i, dst, src, **kw):
                with pin(i):
                    nc.sync.dma_start(dst, src, **kw)

            spdma(0, consts[:], consts_d[:])
            spdma(1, wq_t[:], wqP[:])
            spdma(2, biasf[:], biasf_d[:])
            for q in range(2):
                for g in range(4):
                    cs_ = slice(4 * g, 4 * g + 4)
                    spdma(3 + 4 * q + g, x_q[q][:, cs_, :], xP[:, q, cs_, :])
            spdma(11, wk_t[:], wkP[:])
            spdma(14, wv_t[:], wvP[:])
            spdma(15, x_q[2][:], xP[:, 2, :, :])
            spdma(16, x_q[3][:], xP[:, 3, :, :])
            # ladder slots for the gather loads (emitted later, after their
            # collectives, so the real build keeps correct dependencies):
            KTH_SLOT = {0: (12, 13), 1: (17, 18), 2: (23, 24), 3: (27, 28)}
            VT_SLOT = {0: (19, 20, 21, 22), 1: (25, 26, 29, 30)}

            # ---- PE warmers: lowest scheduler preference, run only when
            # nothing else is ready. Fed from consts so the p-state ramp
            # starts before x arrives and bridges the pre-projection hole. ----
            with tc.high_priority(offset=-200000):
                warm = ps_w.tile([128, 512], F32, tag="w")
                for i in range(32):
                    nc.tensor.matmul(
                        warm[:, 0:128], ones_r[:], ones_r[:],
                        start=True, stop=True, skip_group_check=True)


            # ---- projection helpers ----
            def projq_matmuls(ps, w_sb, q, col, lo, hi):
                # accumulate one seq-quarter of a projection into
                # ps[:, col*256:(col+1)*256]
                cs_ = slice(col * 256, (col + 1) * 256)
                for i in range(lo, hi):
                    nc.tensor.matmul(ps[:, cs_], w_sb[i], x_q[q][:, i, :],
                                     start=(i == 0), stop=(i == NMC - 1))

            def proj2_matmuls(psA, psB, w_sb, h, lo, hi):
                # one PSUM bank per accumulation group (start=True zeroes
                # a full 2KB region)
                for i in range(lo, hi):
                    projq_matmuls(psA, w_sb, 2 * h, 0, i, i + 1)
                    projq_matmuls(psB, w_sb, 2 * h + 1, 0, i, i + 1)

            # bias-add + fp32->bf16 convert on the vector engine (keeps the
            # scalar engine free for the exp stream)
            def k_finish(ps, q, eng=None, on_act=False):
                if on_act:
                    # late K finishes run on the Act engine (idle once the
                    # exp stream ends) so they can't stall the DVE tail
                    # chain; bf16 bias from the consts tile as activation
                    # bias (the proven baseline path)
                    nc.scalar.activation(kq_loc[q][:], ps[:, 0:256], IDENT,
                                         bias=consts[:, 129:130])
                else:
                    nc.vector.tensor_scalar_add(
                        kq_loc[q][:], ps[:, 0:256], bk_sb[:])
                (eng or nc.gpsimd).dma_start(ktd_q[q][:], kq_loc[q][:])

            def k_gather(q):
                if not skip_cc:
                    nc.gpsimd.collective_compute(
                        "AllGather", mybir.AluOpType.bypass,
                        replica_groups=groups,
                        ins=[ktd_q[q].opt()], outs=[ktg_q[q].opt()],
                    )
                s0, s1 = KTH_SLOT[q]
                spdma(s0, kth_q[q][0][:],
                      ktg_q[q][0:2].rearrange("b p c -> p b c"))
                spdma(s1, kth_q[q][1][:],
                      ktg_q[q][2:8].rearrange("b p c -> p b c"))

            def q_finish(psA, psB, h):
                nc.vector.tensor_scalar_add(qt_sb[h][:, 0:256],
                                            psA[:, 0:256], bq_sb[:])
                nc.vector.tensor_scalar_add(qt_sb[h][:, 256:512],
                                            psB[:, 0:256], bq_sb[:])

            def v_finish(psA, psB, h):
                nc.vector.tensor_scalar_add(vt_h[h][:, 0:256],
                                            psA[:, 0:256], bv_sb[:])
                nc.vector.tensor_scalar_add(vt_h[h][:, 256:512],
                                            psB[:, 0:256], bv_sb[:])
                for t in range(4):
                    nc.sync.dma_start(
                        vsb_h[h][:, t, :],
                        vt_h[h][:, t * 128:(t + 1) * 128], transpose=True)
                # half-1's staging runs after the exp stream ends: its
                # vd write can use the then-idle Act queue so it doesn't
                # queue on the bus in front of the tail ktd writes
                eng = nc.scalar if h == 1 else nc.gpsimd
                eng.dma_start(
                    vd_h[h].rearrange("(t p) d -> p t d", p=128),
                    vsb_h[h][:])

            def v_gather(h):
                if not skip_cc:
                    nc.gpsimd.collective_compute(
                        "AllGather", mybir.AluOpType.bypass,
                        replica_groups=groups,
                        ins=[vd_h[h].opt()], outs=[vg_h[h].opt()],
                    )
                for t in range(4):
                    spdma(VT_SLOT[h][t], v_t[h * 4 + t][:],
                          vg_h[h][:, t * 128:(t + 1) * 128, :].rearrange(
                              "b p d -> p b d"))

            # ---- phase A: ALL projections emitted here (so every gather
            # load precedes every score in emission order — the real build's
            # dependencies stay sound), with PRIORITY pins spreading their
            # execution across the slack the exp-paced tile stream leaves.
            # Q0 gates exp-0 and runs at full priority; Q1 must finish
            # before pass 1; everything else is pure PE filler in the timed
            # model. ----
            qt_psA = ps_a.tile([128, 512], F32, tag="pa")
            qt_psB = ps_a.tile([128, 512], F32, tag="pa")
            for g in range(4):
                proj2_matmuls(qt_psA, qt_psB, wq_sb, 0, 4 * g, 4 * g + 4)
            q_finish(qt_psA, qt_psB, 0)

            # tile-priority model: the tile loop starts ~35 unpinned
            # emissions from here; each tile emits ~10 instructions, the
            # pass-0 tail ~20.
            A0 = tc.cur_priority

            def tile_pri(t):
                return A0 + 35 + int(10.2 * t) + (20 if t >= 32 else 0)

            def band(ps_list, w_sb_or_pair, quarters, groups, t0):
                # emit one projection's matmuls pinned to execution tiles
                # t0, t0+1, ... (one chunk-group per tile)
                for i, (lo, hi) in enumerate(groups):
                    with tc.high_priority(
                            offset=tc.cur_priority - tile_pri(t0 + i)):
                        if len(ps_list) == 2:
                            proj2_matmuls(ps_list[0], ps_list[1],
                                          w_sb_or_pair, quarters, lo, hi)
                        else:
                            projq_matmuls(ps_list[0], w_sb_or_pair,
                                          quarters, 0, lo, hi)

            # The ps_a pool is a strict FIFO ring: execution bands must
            # follow allocation order. K0 goes first into the no-PV early
            # slack; the V projections run in pass 0 so their heavy staging
            # (transposes + vd write) never lands on the kernel tail; the
            # K2/K3/K1 chunks (light staging) absorb pass-1's slack.
            G3 = [(0, 6), (6, 11), (11, 16)]
            G5 = [(0, 3), (3, 6), (6, 9), (9, 12), (12, 16)]
            G7 = [(0, 2), (2, 4), (4, 6), (6, 8), (8, 11), (11, 14),
                  (14, 16)]
            G8 = [(0, 2), (2, 4), (4, 6), (6, 8), (8, 10), (10, 12),
                  (12, 14), (14, 16)]
            G11 = [(0, 2), (2, 3), (3, 5), (5, 6), (6, 8), (8, 9), (9, 11),
                   (11, 12), (12, 14), (14, 15), (15, 16)]

            kt_psA = ps_a.tile([128, 512], F32, tag="pa")
            band([kt_psA], wk_sb, 0, G3, 3)
            k_finish(kt_psA, 0)
            k_gather(0)

            qt_psC = ps_a.tile([128, 512], F32, tag="pa")
            qt_psD = ps_a.tile([128, 512], F32, tag="pa")
            band([qt_psC, qt_psD], wq_sb, 1, G11, 8)
            q_finish(qt_psC, qt_psD, 1)

            vt_psA = ps_a.tile([128, 512], F32, tag="pa")
            vt_psB = ps_a.tile([128, 512], F32, tag="pa")
            band([vt_psA, vt_psB], wv_sb, 0, G8, 18)
            v_finish(vt_psA, vt_psB, 0)
            v_gather(0)

            vt_psC = ps_a.tile([128, 512], F32, tag="pa")
            vt_psD = ps_a.tile([128, 512], F32, tag="pa")
            band([vt_psC, vt_psD], wv_sb, 1, G8, 27)
            v_finish(vt_psC, vt_psD, 1)
            v_gather(1)

            kt_psC = ps_a.tile([128, 512], F32, tag="pa")
            band([kt_psC], wk_sb, 2, G8, 36)
            # K2/K3 finish after the exp stream ends: pin their DVE
            # bias-adds behind the pass-1 reciprocal/multiply chain so the
            # in-order DVE queue can't stall the kernel tail
            k_finish(kt_psC, 2, nc.scalar, on_act=True)
            k_gather(2)

            kt_psD = ps_a.tile([128, 512], F32, tag="pa")
            band([kt_psD], wk_sb, 3, G8, 44)
            k_finish(kt_psD, 3, nc.sync, on_act=True)
            k_gather(3)

            # K1 lives in the warm bank's PSUM pool: decoupled from the
            # pa ring, it can fill the early-stream PE idle (needs only
            # x-half0's second quarter + wk) instead of spilling past the
            # last exp.
            kt_psB = ps_w.tile([128, 512], F32, tag="w")
            band([kt_psB], wk_sb, 1, G8, 4)
            k_finish(kt_psB, 1, nc.scalar)
            k_gather(1)

            # ---- phase B: two query-half passes over 32 double-key-block
            # tiles ----
            # quarter-major: tile (h,b,tp) needs exactly K-quarter 2h+tp
            js = [(q // 2, b, q % 2) for q in range(4)
                  for b in range(N_CORES)]
            NT = len(js)

            for u in range(2):
                lag = 10 if u == 0 else 4
                # pass-1's output accumulator lives in the warm bank's pool
                # (free once K1's projection is consumed): its first PV then
                # doesn't wait on pass-0's multiply releasing the ps_o bank
                if u == 0:
                    o_ps = ps_o.tile([128, 512], F32, tag="o")
                else:
                    o_ps = ps_w.tile([128, 512], F32, tag="w")
                acc = fin_pool.tile([128, 2, 512], BF16, name=f"acc{u}")
                pts = {}
                n_acc = 0

                def emit_pv(j, stop=False):
                    h, b, tp = js[j]
                    pt = pts[j]
                    for s in range(2):
                        nc.tensor.matmul(
                            o_ps[:], v_t[h * 4 + 2 * tp + s][:, b, :],
                            pt[:, s, :],
                            start=(j == 0 and s == 0),
                            stop=(stop and s == 1))

                for j in range(NT):
                    h, b, tp = js[j]
                    st = ps_st.tile([128, 2, 512], F32, tag="st")
                    kht = kth_q[2 * h + tp][0 if b < 2 else 1]
                    bi = b if b < 2 else b - 2
                    # boost the first pass-1 scores past the pass-0 tail so
                    # the exp stream doesn't gap at the pass boundary
                    boost = (120 if (u == 1 and j < 2) else
                             40 if (u == 0 and j < 3) else 0)
                    with tc.high_priority(offset=boost):
                        for s in range(2):
                            nc.tensor.matmul(
                                st[:, s, :],
                                kht[:, bi, s * 128:(s + 1) * 128],
                                qt_sb[u][:],
                                start=True, stop=True)
                    pt = pt_pool.tile([128, 2, 512], BF16, tag="pt")
                    nc.scalar.activation(pt[:], st[:], EXP, scale=1.0 / SCALE)
                    pts[j] = pt

                    if j >= lag:
                        emit_pv(j - lag)

                    # bf16 accumulation of exp sums on DVE (pass 1's last
                    # tile is reduced directly by the ones-matmul below;
                    # pass 0 accumulates everything and partition-reduces on
                    # the idle GPSIMD engine)
                    if u == 0 or j < NT - 2:
                        if n_acc == 0:
                            nc.vector.tensor_copy(acc[:], pt[:])
                        else:
                            nc.vector.tensor_add(acc[:], acc[:], pt[:])
                        n_acc += 1

                # pass tail: leftover PVs, denominator, reciprocal, output
                with tc.high_priority(offset=0):
                    rcs = fin_pool.tile([128, 512], F32, name=f"rcs{u}")
                    if u == 0:
                        # pass-0 tail is latency-slack: partition-reduce the
                        # denominators on the idle GPSIMD engine instead of
                        # spending saturated-PE matmuls
                        for j in range(NT - lag, NT - 1):
                            emit_pv(j)
                        emit_pv(NT - 1, stop=True)
                        red = fin_pool.tile([128, 2, 512], F32, name="red0")
                        nc.gpsimd.partition_all_reduce(
                            red[:], acc[:], 128, bass_isa.ReduceOp.add)
                        csum = fin_pool.tile([128, 512], F32, name="csum0")
                        nc.vector.tensor_add(csum[:], red[:, 0, :],
                                             red[:, 1, :])
                        nc.vector.reciprocal(rcs[:], csum[:])
                    else:
                        # pass-1 tail is the kernel tail: keep the fast
                        # PE ones-matmul reduction. The acc part runs before
                        # the last exp completes; the pt_last part joins the
                        # same PSUM group, emitted before the last PV so the
                        # reciprocal chain starts ASAP. cs lives in the st
                        # ring (its slot frees right at the pass tail).
                        # acc's partition reduction runs on GPSIMD before
                        # the last exp (saving saturated-PE matmuls); only
                        # the last tile's sums use the fast PE path
                        red1 = fin_pool.tile([128, 2, 512], F32,
                                             name="red1")
                        nc.gpsimd.partition_all_reduce(
                            red1[:], acc[:], 128, bass_isa.ReduceOp.add)
                        rsum = fin_pool.tile([128, 512], F32, name="rsum1")
                        nc.vector.tensor_add(rsum[:], red1[:, 0, :],
                                             red1[:, 1, :])
                        cs_t = ps_st.tile([128, 512], F32, tag="st")
                        cs = cs_t[:]
                        srcs = [pts[NT - 2][:, 0, :], pts[NT - 2][:, 1, :],
                                pts[NT - 1][:, 0, :], pts[NT - 1][:, 1, :]]
                        nc.tensor.matmul(cs[:], ones_r[:], srcs[0],
                                         start=True, stop=False)
                        nc.tensor.matmul(cs[:], ones_r[:], srcs[1],
                                         start=False, stop=False)
                        for j in range(NT - lag, NT - 1):
                            emit_pv(j)
                        nc.tensor.matmul(cs[:], ones_r[:], srcs[2],
                                         start=False, stop=False)
                        nc.tensor.matmul(cs[:], ones_r[:], srcs[3],
                                         start=False, stop=True)
                        emit_pv(NT - 1, stop=True)
                        csum = fin_pool.tile([128, 512], F32, name="csum1")
                        nc.vector.tensor_add(csum[:], cs[:], rsum[:])
                        nc.vector.reciprocal(rcs[:], csum[:])
                    nc.vector.tensor_mul(rcs[:], o_ps[:], rcs[:])
                    nc.sync.dma_start(out[:, u * 512:(u + 1) * 512], rcs[:])
                pts.clear()

    nc.compile()
    return nc


_NC_CACHE = {}


def _get_nc():
    key = os.environ.get("KCC", "")
    if key not in _NC_CACHE:
        _NC_CACHE[key] = _build()
    return _NC_CACHE[key]


def _run(inputs, trace=False, **spmd_kwargs):
    BF = ml_dtypes.bfloat16
    x = np.asarray(inputs["x"], dtype=np.float32)
    Wq = np.asarray(inputs["Wq"], dtype=np.float32)
    Wk = np.asarray(inputs["Wk"], dtype=np.float32)
    Wv = np.asarray(inputs["Wv"], dtype=np.float32)
    bq = np.asarray(inputs["bq"], dtype=np.float32)
    bk = np.asarray(inputs["bk"], dtype=np.float32)
    bv = np.asarray(inputs["bv"], dtype=np.float32)

    def prepack(wT):
        # [DM, DK] -> [128, NMC, DK]: partition p, chunk c holds row c*128+p
        return np.ascontiguousarray(
            wT.reshape(NMC, 128, -1).transpose(1, 0, 2)).astype(BF)

    consts = np.zeros((128, 132), dtype=np.float32)
    consts[:, 0:128] = 1.0
    consts[:, 128] = bq
    consts[:, 129] = bk
    consts[:, 130] = bv
    biasf = np.zeros((128, 3), dtype=np.float32)
    biasf[:, 0] = bq
    biasf[:, 1] = bk
    biasf[:, 2] = bv
    shared = {
        "wqP": prepack(Wq.T),
        "wkP": prepack(Wk.T),
        "wvP": prepack(Wv.T),
        "consts_d": consts.astype(BF),
        "biasf_d": biasf,
    }
    in_maps = []
    for c in range(N_CORES):
        xT_c = np.ascontiguousarray(x[c * SL:(c + 1) * SL].T)
        # [128 part, 4 seq-quarter, 16 chunk, 256]: row c*128+p, col q*256+s
        xq = np.ascontiguousarray(
            xT_c.reshape(NMC, 128, 4, 256).transpose(1, 2, 0, 3)).astype(BF)
        in_maps.append({"xP": xq, **shared})

    nc = _get_nc()
    res = run_bass_kernel_spmd(
        nc, in_maps, core_ids=list(range(N_CORES)), trace=trace, **spmd_kwargs)
    full = np.concatenate(
        [np.ascontiguousarray(res.results[c]["out"].T)
         for c in range(N_CORES)], axis=0)
    return full, res


def kernel(**inputs):
    out, _ = _run(inputs)
    return out
